# revision 52
# baseline (speedup 1.0000x reference)
"""Block-diagonal linear y = x @ W_blockdiag.T + bias on 8 TRN2 NeuronCores.

Expert-parallel sharding: core k owns diagonal block k -- x[:, 512k:512(k+1)],
weight_blocks[k] (512x512), bias[512k:512(k+1)] -- and produces the matching
output column slice y[:, 512k:512(k+1)]. No collectives.

This problem sits at the roofline ridge in 16-bit: per core the HBM floor is
(8 MiB x + 8 MiB y + 0.5 MiB W) / 358 GB/s ~= 46 us and the PE MAC floor is
8192*512*512 MACs / (128*128/cyc) = 131072 cyc ~= 55 us. The kernel therefore
keeps the PE stream free of everything except the 256 mandatory matmuls:

  - fp16 everywhere on the wire (tolerance is 2e-2; fp16 lands ~4e-4)
  - x is uploaded pre-transposed and pre-tiled per core as xT[p, ci, n]
    (= x[n, ci*128+p]), so no PE/DMA transposes are needed on device
  - compute yT[r, n] = sum_c W[r, c] xT[c, n]: stationary lhsT = 128x128
    blocks of W^T (16 of them, resident in SBUF the whole kernel), moving
    rhs = xT token chunks, free dim 512 = one full PSUM bank per matmul
  - bias add fused into the PSUM->SBUF evacuation (per-partition scalar,
    alternating DVE tensor_scalar / ACT activation-Identity), output cast
    to fp16 in the same op
  - yT stored as [p, rj, n]; the host un-transposes both directions
  - x loads on the SP HWDGE ring; W/bias/y stores on the ACT HWDGE ring,
    with W split across both rings at startup (cold-ring first-packet
    latency is ~2-3.5 us and run-to-run variable, so the first-matmul
    dependencies ride max-of-rings, not sum)
  - PE warm-up burst of dummy transposes (reading a memset scratch tile,
    no DMA dependency) spans the ~3.4 us HAM activity window right after
    the runtime preamble, so real matmuls start at the full 2.4 GHz clock
  - tail: the last sub-chunk's evac is split across DVE+ACT and its y
    strips are stored per-rj with the issues spread over both sequencers,
    so the end-of-kernel critical path is one small 64 KiB store
"""

import os
import sys

import numpy as np

for _p in ("/opt/trn_rl_repo", "/root/.axon_site/_ro/trn_rl_repo"):
    if os.path.isdir(_p) and _p not in sys.path:
        sys.path.insert(0, _p)

import concourse.bass as bass
import concourse.mybir as mybir
import concourse.tile as tile
from concourse.bass_utils import run_bass_kernel_spmd
from concourse.tile_rust import add_dep_helper

# Problem shape (hardcoded per spec nn_BlockDiagLinear_19490561590005)
N = 8192          # tokens
D = 4096          # model dim
NB = 8            # diagonal blocks == number of cores
B = 512           # block size (rows == cols)
P = 128           # SBUF partitions
CB = B // P       # 4 contraction chunks of 128
RB = B // P       # 4 output-row chunks of 128
TS = 512          # tokens per sub-chunk == PSUM bank free size (fp32)
SUBS = N // TS    # 16 sub-chunks

F32 = mybir.dt.float32
F16 = mybir.dt.float16

# sub-chunks per DMA chunk. Small first chunks = fast pipeline fill; small
# last chunks = short store tail. One sub-chunk = 256 KiB of x traffic.
SCHED = [1, 1, 2, 4, 4, 2, 1, 1]
assert sum(SCHED) == SUBS
PRELOAD_CHUNKS = 3
WARMUP_TRANSPOSES = 20  # dummy fp32 transposes -> HAM at 8/8 for real MMs;
                        # sized to end ~when the first x/W bytes land
                        # (x0/W are few-descriptor DMAs now; the remaining
                        # variable is the ~1-3.5 us HWDGE ring bring-up)

_CACHE = {}


def _build_bass():
    nc = bass.Bass("TRN2", target_bir_lowering=False)
    # host-prearranged layouts (see _run): all fp16, partition-major. x and y
    # are arranged CHUNK-MAJOR, i.e. exactly the SBUF tile layout of each
    # SCHED chunk concatenated, so every chunk DMA is 128 fully-contiguous
    # descriptors (one per partition). The HWDGE generates descriptors at
    # ~7 ns each, so descriptor count -- not HBM bandwidth -- is what sets
    # both the first-chunk arrival time and the tail store drain.
    x_d = nc.dram_tensor("x", [P, CB * N], F16, kind="ExternalInput")
    w_d = nc.dram_tensor("w", [P, CB * B], F16, kind="ExternalInput")
    b_d = nc.dram_tensor("b", [P, RB], F32, kind="ExternalInput")
    y_d = nc.dram_tensor("y", [P, RB * N], F16, kind="ExternalOutput")

    with tile.TileContext(nc) as tc:
        with (
            tc.tile_pool(name="const", bufs=1) as const_pool,
            tc.tile_pool(name="xin", bufs=5) as x_pool,
            tc.tile_pool(name="yout", bufs=4) as y_pool,
            tc.tile_pool(name="psY", bufs=7, space="PSUM") as psY_pool,
            tc.tile_pool(name="psDummy", bufs=1, space="PSUM") as psD_pool,
        ):
            chunk_of = {}
            acc = 0
            for g in SCHED:
                chunk_of[acc] = g
                acc += g

            def load_x_chunk(s, g, eng=None):
                # xbig[p, ci*g*TS + j] = x[s*TS+j, ci*128+p]; the DRAM region
                # is the identical layout, so this is one 128-descriptor DMA.
                # Chunk 0 is split into per-ci quarter DMAs: the first matmul
                # only needs ci=0, so the stream starts at quarter-1-ready
                # (~1.5 us earlier when data-gated) while later quarters
                # arrive just-in-time for their ci's matmuls.
                x_big = x_pool.tile([P, CB * g * TS], F16, tag="xbig")
                if s == 0:
                    for ci in range(CB):
                        nc.sync.dma_start(
                            out=x_big[:, ci * g * TS : (ci + 1) * g * TS],
                            in_=x_d.ap()[:, ci * g * TS : (ci + 1) * g * TS],
                        )
                    return x_big
                (eng or nc.sync).dma_start(
                    out=x_big,
                    in_=x_d.ap()[:, CB * TS * s : CB * TS * (s + g)],
                )
                return x_big

            # Startup DMAs. The first packet on a cold HWDGE ring takes
            # ~2-3.5 us (which ring comes up fast is run-to-run variable), so
            # the two first-matmul dependencies go on SEPARATE rings -- x
            # chunk 0 first on SP, the W blocks (rj=0 first) + bias on ACT --
            # making the critical path max(), not sum(), of ring latencies.
            w_sb = const_pool.tile([P, RB * B], F16)
            b_sb = const_pool.tile([P, RB], F32)
            preloaded = {}
            def load_w(rj, eng):
                eng.dma_start(
                    out=w_sb[:, rj * B : (rj + 1) * B],
                    in_=w_d.ap()[:, rj * B : (rj + 1) * B],
                )

            with tc.high_priority():
                preloaded[0] = load_x_chunk(0, chunk_of[0])
                load_w(0, nc.scalar)
                load_w(1, nc.scalar)
                load_w(2, nc.sync)   # behind x0 on the SP ring
                load_w(3, nc.sync)
                nc.scalar.dma_start(out=b_sb, in_=b_d.ap())
                for s in sorted(chunk_of)[1:PRELOAD_CHUNKS]:
                    preloaded[s] = load_x_chunk(s, chunk_of[s])

            # PE warm-up burst: transposes of a memset scratch tile (values
            # are irrelevant -- the PSUM output is never read). The only
            # upstream dependency is a GpSimd memset, so the burst starts
            # right after the runtime preamble barrier releases the engines,
            # ~3.4 us (one HAM activity window) before the first x/W bytes
            # can possibly land.
            scratch = const_pool.tile([P, P], F32)
            nc.vector.memset(scratch, 0.0)
            ps_dummy = psD_pool.tile([P, P], F32, tag="tail")
            dummy_inst = nc.tensor.transpose(ps_dummy, scratch, scratch)
            for _ in range(WARMUP_TRANSPOSES - 1):
                nc.tensor.transpose(ps_dummy, scratch, scratch)

            # main loop over 16 token sub-chunks, DMA-chunked per SCHED
            x_big = None
            y_big = None
            base = 0
            first_mm = None
            for s in range(SUBS):
                if s in chunk_of:
                    g = chunk_of[s]
                    base = s
                    x_big = preloaded.pop(s, None)
                    if x_big is None:
                        x_big = load_x_chunk(s, g)
                    y_big = y_pool.tile([P, RB * g * TS], F16, tag="ybig")

                g = chunk_of[base]
                off = (s - base) * TS  # token offset within the chunk
                last_sub = base + g == SUBS and s - base == g - 1
                # (splitting the final rj into two 256-token halves so the
                # first half's evac+store overlap the second half's matmuls
                # was tried and measured SLOWER: the drain's serial DMA-lane
                # observation and the extra store issue eat the gain)
                for rj in range(RB):
                    a = rj * g * TS + off
                    psy = psY_pool.tile([P, TS], F32)
                    for ci in range(CB):
                        mm = nc.tensor.matmul(
                            psy,
                            w_sb[:, rj * B + ci * P : rj * B + (ci + 1) * P],
                            x_big[:, ci * g * TS + off : ci * g * TS + off + TS],
                            start=(ci == 0),
                            stop=(ci == CB - 1),
                        )
                        if first_mm is None:
                            first_mm = mm
                            add_dep_helper(
                                mm.ins, dummy_inst.ins, sync=False,
                                reason="warmup before first matmul",
                            )
                    # fused bias add + fp32->fp16 cast + PSUM->SBUF
                    # evacuation, alternating DVE / ACT
                    dst = y_big[:, a : a + TS]
                    if (s * RB + rj) % 2 == 0:
                        nc.vector.tensor_scalar_add(dst, psy, b_sb[:, rj : rj + 1])
                    else:
                        nc.scalar.add(dst, psy, b_sb[:, rj : rj + 1])

                if s - base == g - 1:
                    yoff = RB * TS * base
                    if last_sub:
                        # tail: per-rj stores, everything except rj3's final
                        # half on the idle Sync ring (those evacs complete
                        # while later matmuls still run); rj3's half B on the
                        # scalar ring right after its ACT evac -- the issues
                        # overlap across the two sequencers and the final
                        # transfer is only 32 KiB.
                        for rj in range(RB):
                            eng = nc.scalar if rj == RB - 1 else nc.sync
                            eng.dma_start(
                                out=y_d.ap()[
                                    :, yoff + rj * g * TS : yoff + (rj + 1) * g * TS
                                ],
                                in_=y_big[:, rj * g * TS : (rj + 1) * g * TS],
                            )
                    else:
                        # (routing the penultimate chunk's store to the SP
                        # ring was tried and is NOT faster: it unbalances the
                        # two rings' tail drain)
                        nc.scalar.dma_start(
                            out=y_d.ap()[:, yoff : yoff + RB * g * TS],
                            in_=y_big,
                        )

    return nc


def _split_pe_multiwaits(nc):
    """Hoist extra sync waits off engine instructions onto sequencer NoOps.

    This walrus build supports only a single attached sync wait per
    instruction; codegen fails with "Too many sync wait commands" otherwise.
    A wait-carrying NoOp immediately before the instruction on the same
    sequencer is semantically identical (the sequencer executes in order).
    """
    k = 0
    for f in nc.m.functions:
        for blk in f.blocks:
            out = []
            changed = False
            for inst in blk.instructions:
                si = inst.sync_info
                if si is not None and len(si.on_wait) > 1:
                    waits = list(si.on_wait)
                    for w in waits[:-1]:
                        nop = mybir.InstNoOp(
                            name=f"I-waitsplit-{k}", ins=[], outs=[]
                        )
                        k += 1
                        nop.engine = inst.engine
                        nop.sync_info = mybir.SyncInfo(on_wait=[w], on_update=[])
                        out.append(nop)
                    inst.sync_info = mybir.SyncInfo(
                        on_wait=[waits[-1]], on_update=list(si.on_update)
                    )
                    changed = True
                out.append(inst)
            if changed:
                blk.instructions = out
    return nc


def _get_nc():
    if "nc" not in _CACHE:
        _CACHE["nc"] = _split_pe_multiwaits(_build_bass())
    return _CACHE["nc"]


def _chunks():
    out = []
    base = 0
    for g in SCHED:
        out.append((base, g))
        base += g
    return out


def _shard_inputs(x, w, bias):
    """Slice per core and prearrange into the device layouts (all fp16)."""
    in_maps = []
    for k in range(NB):
        xk = x[:, k * B : (k + 1) * B].astype(np.float16)  # [N, 512]
        # chunk-major: for each SCHED chunk, the exact SBUF tile layout
        # xbig[p, ci*g*TS + j] = x[base*TS + j, ci*128 + p]
        xTc = xk.T.reshape(CB, P, N)  # [ci, p, n]
        xT = np.concatenate(
            [
                xTc[:, :, base * TS : (base + g) * TS]
                .transpose(1, 0, 2)
                .reshape(P, CB * g * TS)
                for base, g in _chunks()
            ],
            axis=1,
        )  # [P, CB*N]
        wk = w[k].astype(np.float16)                        # [r, c] = [512, 512]
        # w_arr[p, rj*512 + ci*128 + rl] = W[rj*128+rl, ci*128+p]
        # (rj-major so the first DMA covers everything the rj=0 matmuls need)
        w_arr = np.ascontiguousarray(
            wk.reshape(RB, P, CB, P).transpose(3, 0, 2, 1).reshape(P, RB * CB * P)
        )
        bk = bias[k * B : (k + 1) * B].astype(np.float32)
        b_arr = np.ascontiguousarray(bk.reshape(RB, P).T)   # [p, rj]
        in_maps.append({"x": xT, "w": w_arr, "b": b_arr})
    return in_maps


def _unshard_output(res):
    # y_d is chunk-major: y_d[p, RB*TS*base + rj*g*TS + j] = y[base*TS+j,
    # rj*128+p] per core; invert per chunk and concat cores.
    cols = []
    for k in range(NB):
        yT = np.asarray(res.results[k]["y"], dtype=np.float32)  # [P, RB*N]
        yk = np.empty((N, B), dtype=np.float32)
        for base, g in _chunks():
            blk = yT[:, RB * TS * base : RB * TS * (base + g)]
            yk[base * TS : (base + g) * TS, :] = (
                blk.reshape(P, RB, g * TS).transpose(2, 1, 0).reshape(g * TS, B)
            )
        cols.append(yk)
    return np.ascontiguousarray(np.concatenate(cols, axis=1))


def _run(inputs, trace=False):
    x = np.ascontiguousarray(np.asarray(inputs["x"], dtype=np.float32))
    w = np.ascontiguousarray(np.asarray(inputs["weight_blocks"], dtype=np.float32))
    bias = np.ascontiguousarray(np.asarray(inputs["bias"], dtype=np.float32))
    assert x.shape == (N, D) and w.shape == (NB, B, B) and bias.shape == (D,)
    nc = _get_nc()
    in_maps = _shard_inputs(x, w, bias)
    try:
        res = run_bass_kernel_spmd(
            nc, in_maps, core_ids=list(range(NB)), trace=trace
        )
    except Exception:
        # the axon-tunneled devices occasionally report a transient
        # NRT_EXEC_UNIT_UNRECOVERABLE; a single retry has always recovered
        res = run_bass_kernel_spmd(
            nc, in_maps, core_ids=list(range(NB)), trace=trace
        )
    return _unshard_output(res), res


def kernel(**inputs):
    y, _ = _run(inputs, trace=False)
    return y


def kernel_traced(**inputs):
    return _run(inputs, trace=True)
